# revision 16
# baseline (speedup 1.0000x reference)
"""EntityMemory Trainium2 kernel (8 NeuronCores, entity-sharded).

Strategy:
  - Host: BIO span-end scan + compaction to BEGIN tokens only (~1/3 of B*S),
    builds span features A=[X[t], X[end(t)]] and per-core shards of the
    entity table E_w over N (model parallel, 6250 entities/core, padded 6272).
  - Device (SPMD, 8 cores): pseudo_T = Wf @ A_T; scores_T = E_shard.T @ pseudo_T
    per 128-entity chunk; exp on ACT (fp16); picked_T += E_shT.T @ exp (PSUM
    accum); softmax denominator accumulated on DVE (exp-sum) + one
    ones-matmul partition reduce; loss gather via host-gathered E columns
    (pseudo . E_sel) folded into the same ReduceScatter payload.
  - Tokens are processed in NSPLIT interleaved slices (each slice holds the
    same sub-range of every rank's tokens), and each slice gets its own
    bf16 ReduceScatter - earlier collectives overlap later compute.
  - Post-RS each core normalizes its token slice, applies Wb (+bias via a
    K=1 matmul row) and writes its y slice + loss partial.
  - Host: scatter y rows back to BEGIN positions, sum loss partials.

All device inputs are host-preswizzled to the exact [128, free] SBUF layout
so every input DMA is a contiguous partition-major copy.
"""

import sys

sys.path.insert(0, "/opt/trn_rl_repo")

import numpy as np
import ml_dtypes

B, S, D_EMB, D_ENT, N = 4, 512, 768, 256, 50000
BEGIN, INNER = 1, 2
NCORES = 8
SHARD = N // NCORES               # 6250 entities per core
NCH = (SHARD + 127) // 128        # 49 chunks
NS = NCH * 128                    # 6272 padded shard width
KA = (2 * D_EMB) // 128           # 12 contraction chunks for Wf
PAD_ENT = float(NCORES * NS - N)  # 176.0 pad entities, each contributing exp(0)=1

TRACE = False
LAST_RESULTS = None

_cache = {}


def _build(T):
    import concourse.mybir as mybir
    import concourse.tile as tile
    from concourse import bacc
    from concourse.tile_rust import add_dep_helper

    dt = mybir.dt
    f32, bf16, f16 = dt.float32, dt.bfloat16, dt.float16
    AF = mybir.ActivationFunctionType
    TS = T // NCORES
    MT = (TS + 127) // 128          # m-tiles over the final token slice
    NSPLIT = 2 if T <= 1024 else T // 512
    HTS = TS // NSPLIT              # per-rank tokens per slice
    NSL = NCORES * HTS              # matmul N per slice

    nc = bacc.Bacc("TRN2", target_bir_lowering=False, debug=False, num_devices=NCORES)

    a_t = nc.dram_tensor("a_t", [128, KA * T], bf16, kind="ExternalInput")
    wf_t = nc.dram_tensor("wf_t", [128, KA * D_ENT], bf16, kind="ExternalInput")
    wf_b = nc.dram_tensor("wf_b", [128, 2], f32, kind="ExternalInput")
    e_sh = nc.dram_tensor("e_sh", [128, 2 * NS], bf16, kind="ExternalInput")
    e_sht = nc.dram_tensor("e_sht", [128, NCH * D_ENT], f16, kind="ExternalInput")
    e_sel = nc.dram_tensor("e_sel", [128, 2 * T], bf16, kind="ExternalInput")
    mask = nc.dram_tensor("mask", [1, TS], f32, kind="ExternalInput")
    wb_t = nc.dram_tensor("wb_t", [128, 2 * D_EMB], bf16, kind="ExternalInput")
    wb_b = nc.dram_tensor("wb_b", [1, D_EMB], bf16, kind="ExternalInput")
    y_out = nc.dram_tensor("y_out", [TS, D_EMB], f32, kind="ExternalOutput")
    loss_out = nc.dram_tensor("loss_out", [1, 1], f32, kind="ExternalOutput")

    def r3(ap, j, s):
        """Interleaved-slice view: [128, 8, HTS] of column block j, slice s."""
        return ap[:, j * T : (j + 1) * T].rearrange("p (r w) -> p r w", r=NCORES)[
            :, :, s * HTS : (s + 1) * HTS
        ]

    with tile.TileContext(nc) as tc:
        with (
            tc.tile_pool(name="res", bufs=1) as res,
            tc.tile_pool(name="work", bufs=3) as work,
            tc.tile_pool(name="exl", bufs=8) as exl,
            tc.tile_pool(name="dram", bufs=1, space="DRAM") as dram,
        ):
            # ---- resident SBUF tiles (contiguous chunked DMAs) ----
            a_sb = res.tile([128, KA * T], bf16)
            wf_sb = res.tile([128, KA * D_ENT], bf16)
            wfb_sb = res.tile([128, 2], f32)
            esh_sb = res.tile([128, 2 * NS], bf16)
            esht_sb = res.tile([128, NCH * D_ENT], f16)
            esel_sb = res.tile([128, 2 * T], bf16)
            mask_sb = res.tile([1, TS], f32)
            wbt_sb = res.tile([128, 2 * D_EMB], bf16)
            wbb_sb = res.tile([1, D_EMB], bf16)
            ones32 = res.tile([128, 1], f32)
            onesrow = res.tile([1, TS], bf16)
            pseudo_sb = res.tile([128, 2 * T], bf16)
            prod_sb = res.tile([128, 2 * T], f32)
            acc_sb = res.tile([128, T], f32)

            nc.vector.memset(ones32, 1.0)
            nc.vector.memset(onesrow, 1.0)
            nc.vector.memset(acc_sb, 0.0)

            # pseudo-phase inputs first, interleaved so matmul k can start as
            # soon as chunk k lands
            for kk in range(KA):
                nc.sync.dma_start(
                    out=a_sb[:, kk * T : (kk + 1) * T],
                    in_=a_t[:, kk * T : (kk + 1) * T],
                )
                nc.sync.dma_start(
                    out=wf_sb[:, kk * D_ENT : (kk + 1) * D_ENT],
                    in_=wf_t[:, kk * D_ENT : (kk + 1) * D_ENT],
                )
            nc.sync.dma_start(out=wfb_sb, in_=wf_b[:, :])
            EGRP = 7
            for c0 in range(0, NCH, EGRP):
                w = min(EGRP, NCH - c0) * 128
                for j in range(2):
                    nc.sync.dma_start(
                        out=esh_sb[:, j * NS + c0 * 128 : j * NS + c0 * 128 + w],
                        in_=e_sh[:, j * NS + c0 * 128 : j * NS + c0 * 128 + w],
                    )
                w = min(EGRP, NCH - c0) * D_ENT
                nc.sync.dma_start(
                    out=esht_sb[:, c0 * D_ENT : c0 * D_ENT + w],
                    in_=e_sht[:, c0 * D_ENT : c0 * D_ENT + w],
                )
            nc.sync.dma_start(out=esel_sb, in_=e_sel[:, :])
            nc.sync.dma_start(out=mask_sb, in_=mask[:, :])
            nc.sync.dma_start(out=wbt_sb, in_=wb_t[:, :])
            nc.sync.dma_start(out=wbb_sb, in_=wb_b[:, :])

            rs_ins = [dram.tile([NCORES, 258 * HTS], bf16, tag=f"rsi{s}", name=f"rs_in{s}") for s in range(NSPLIT)]
            rs_outs = [dram.tile([NCORES, 258 * HTS], bf16, tag=f"rso{s}", name=f"rs_out{s}") for s in range(NSPLIT)]

            # ---- phase 1: pseudo_T[d_ent, tok] = Wf @ A_T + Wf_b ----
            with tc.tile_pool(name="pp", bufs=2, space="PSUM") as pp:
                for j in range(2):
                    for n0 in range(0, T, 512):
                        nsl = min(512, T - n0)
                        ps = pp.tile([128, 512], f32)
                        for kk in range(KA):
                            nc.tensor.matmul(
                                ps[:, :nsl],
                                wf_sb[:, kk * D_ENT + j * 128 : kk * D_ENT + (j + 1) * 128],
                                a_sb[:, kk * T + n0 : kk * T + n0 + nsl],
                                start=(kk == 0),
                                stop=(kk == KA - 1),
                            )
                        nc.vector.tensor_scalar_add(
                            pseudo_sb[:, j * T + n0 : j * T + n0 + nsl],
                            ps[:, :nsl],
                            wfb_sb[:, j : j + 1],
                        )

            # ---- loss-gather products: prod = pseudo * E_sel ----
            for j in range(2):
                nc.vector.tensor_mul(
                    prod_sb[:, j * T : (j + 1) * T],
                    pseudo_sb[:, j * T : (j + 1) * T],
                    esel_sb[:, j * T : (j + 1) * T],
                )

            # ---- phase 2: entity-chunk loop per interleaved slice ----
            with (
                tc.tile_pool(name="sp", bufs=4, space="PSUM") as sp,
                tc.tile_pool(name="ac", bufs=1, space="PSUM") as ac,
            ):
                prev_coll = None
                all_scatters = []
                for s in range(NSPLIT):
                    pk0 = ac.tile([128, 512], f32, tag="pk0")
                    pk1 = ac.tile([128, 512], f32, tag="pk1")
                    pks = [pk0, pk1]
                    for c in range(NCH):
                        sc = sp.tile([128, 512], f32)
                        for j in range(2):
                            nc.tensor.matmul(
                                sc[:, :NSL],
                                esh_sb[:, j * NS + c * 128 : j * NS + (c + 1) * 128],
                                r3(pseudo_sb, j, s),
                                start=(j == 0),
                                stop=(j == 1),
                            )
                        ex = exl.tile([128, 512], f16, tag="ex")
                        nc.scalar.activation(ex[:, :NSL], sc[:, :NSL], AF.Exp)
                        for m in range(2):
                            nc.tensor.matmul(
                                pks[m][:, :NSL],
                                esht_sb[:, c * D_ENT + m * 128 : c * D_ENT + (m + 1) * 128],
                                ex[:, :NSL],
                                start=(c == 0),
                                stop=(c == NCH - 1),
                            )
                        # denominator partial on DVE (keeps PE free)
                        nc.vector.tensor_add(
                            r3(acc_sb, 0, s),
                            r3(acc_sb, 0, s),
                            ex[:, :NSL].rearrange("p (r w) -> p r w", r=NCORES),
                        )
                    # partition-reduce of exp-sum + gather row for this slice
                    den = ac.tile([1, 512], f32, tag="den")
                    gps = ac.tile([1, 512], f32, tag="gps")
                    nc.tensor.matmul(
                        den[:1, :NSL], ones32[:, 0:1], r3(acc_sb, 0, s),
                        start=True, stop=True,
                    )
                    for j in range(2):
                        nc.tensor.matmul(
                            gps[:1, :NSL], ones32[:, 0:1], r3(prod_sb, j, s),
                            start=(j == 0), stop=(j == 1),
                        )
                    # copy partials out and scatter into RS payload blocks.
                    # dst view [d, r, w]: block r at r*258*HTS, row d at d*HTS.
                    dst3 = rs_ins[s][:, :].rearrange("r (d w) -> d r w", d=258)
                    scatter_dmas = []
                    for m in range(2):
                        pk_sb = work.tile([128, 512], bf16, tag="pk_sb")
                        nc.vector.tensor_copy(pk_sb[:, :NSL], pks[m][:, :NSL])
                        scatter_dmas.append(nc.sync.dma_start(
                            out=dst3[m * 128 : (m + 1) * 128, :, :],
                            in_=pk_sb[:, :NSL].rearrange("p (r w) -> p r w", r=NCORES),
                        ))
                    den_sb = work.tile([1, 512], bf16, tag="den_sb")
                    g_sb = work.tile([1, 512], bf16, tag="g_sb")
                    nc.vector.tensor_copy(den_sb[:1, :NSL], den[:1, :NSL])
                    nc.vector.tensor_copy(g_sb[:1, :NSL], gps[:1, :NSL])
                    scatter_dmas.append(nc.sync.dma_start(
                        out=dst3[256:257, :, :],
                        in_=den_sb[:1, :NSL].rearrange("p (r w) -> p r w", r=NCORES),
                    ))
                    scatter_dmas.append(nc.sync.dma_start(
                        out=dst3[257:258, :, :],
                        in_=g_sb[:1, :NSL].rearrange("p (r w) -> p r w", r=NCORES),
                    ))
                    # keep later slices' scatter DMAs ordered after this
                    # slice's collective so its queue-sem waits stay tight
                    all_scatters.extend(scatter_dmas)
                    if prev_coll is not None:
                        for dma_i in scatter_dmas:
                            add_dep_helper(
                                dma_i.ins, prev_coll.ins, sync=False,
                                reason="scatter after prior slice RS",
                            )
                    # slice-s ReduceScatter: overlaps the next slice's compute
                    coll = nc.gpsimd.collective_compute(
                        "AllToAll",
                        mybir.AluOpType.bypass,
                        replica_groups=[list(range(NCORES))],
                        ins=[rs_ins[s][:, :].opt()],
                        outs=[rs_outs[s][:, :].opt()],
                    )
                    prev_coll = coll

            # ---- phase 4 (per slice): local 8-way reduce of A2A blocks,
            # normalize, down-project, write y slice + loss partial ----
            ones_bf = res.tile([128, 1], bf16)
            nc.vector.memset(ones_bf, 1.0)
            pick_bf = res.tile([128, 2 * TS], bf16)
            den_rf = res.tile([1, TS], f32)
            recip_row = res.tile([1, TS], f32)
            loss_acc = res.tile([1, NSPLIT], f32)
            post_reads = []
            with (
                tc.tile_pool(name="fp", bufs=2, space="PSUM") as fp,
                tc.tile_pool(name="fw", bufs=2) as fw,
            ):
                for s in range(NSPLIT):
                    # src view [d, i, w]: contributor i's row d for my tokens
                    src3 = rs_outs[s][:, :].rearrange("i (d w) -> d i w", d=258)
                    for m in range(2):
                        blk = fw.tile([128, NCORES * HTS], bf16, tag="blk")
                        post_reads.append(nc.sync.dma_start(
                            out=blk.rearrange("p (i w) -> p i w", i=NCORES),
                            in_=src3[m * 128 : (m + 1) * 128, :, :],
                        ))
                        pacc = fw.tile([128, HTS], f32, tag="pacc")
                        nc.vector.tensor_add(
                            pacc, blk[:, 0:HTS], blk[:, HTS : 2 * HTS]
                        )
                        for i in range(2, NCORES):
                            nc.vector.tensor_add(
                                pacc, pacc, blk[:, i * HTS : (i + 1) * HTS]
                            )
                        nc.vector.tensor_copy(
                            pick_bf[:, m * TS + s * HTS : m * TS + (s + 1) * HTS], pacc
                        )
                    dg = fw.tile([8, 2 * HTS], bf16, tag="dg")
                    post_reads.append(nc.sync.dma_start(
                        out=dg[:, 0:HTS], in_=rs_outs[s][:, 256 * HTS : 257 * HTS]
                    ))
                    post_reads.append(nc.sync.dma_start(
                        out=dg[:, HTS : 2 * HTS], in_=rs_outs[s][:, 257 * HTS : 258 * HTS]
                    ))
                    den_ps = fp.tile([1, 512], f32, tag="den_ps")
                    g_ps = fp.tile([1, 512], f32, tag="g_ps")
                    nc.tensor.matmul(
                        den_ps[0:1, :HTS], ones_bf[0:8, 0:1], dg[:, 0:HTS],
                        start=True, stop=True,
                    )
                    nc.tensor.matmul(
                        g_ps[0:1, :HTS], ones_bf[0:8, 0:1], dg[:, HTS : 2 * HTS],
                        start=True, stop=True,
                    )
                    # denominator (minus zero-pad entity contribution) + recip
                    nc.vector.tensor_scalar_add(
                        den_rf[:, s * HTS : (s + 1) * HTS], den_ps[0:1, :HTS], -PAD_ENT
                    )
                    nc.vector.reciprocal(
                        recip_row[:, s * HTS : (s + 1) * HTS],
                        den_rf[:, s * HTS : (s + 1) * HTS],
                    )
                    # loss partial: sum(mask * exp(g) / den) over this slice
                    g_exp = fw.tile([1, HTS], f32, tag="g_exp")
                    nc.scalar.activation(g_exp, g_ps[0:1, :HTS], AF.Exp)
                    nc.vector.tensor_mul(
                        g_exp, g_exp, recip_row[:, s * HTS : (s + 1) * HTS]
                    )
                    nc.vector.tensor_mul(
                        g_exp, g_exp, mask_sb[:, s * HTS : (s + 1) * HTS]
                    )
                    nc.vector.reduce_sum(
                        loss_acc[:, s : s + 1], g_exp, axis=mybir.AxisListType.X
                    )
                    # per-token reciprocal column: transposed view [w, i] of
                    # the 8 contributors' den rows, reduced along free dim
                    dgT = fw.tile([128, NCORES], bf16, tag="dgT")
                    post_reads.append(nc.sync.dma_start(
                        out=dgT[0:HTS, :],
                        in_=rs_outs[s][:, 256 * HTS : 257 * HTS].rearrange(
                            "i w -> w i"
                        ),
                    ))
                    den_cs = fw.tile([128, 1], f32, tag="den_cs")
                    recip_s = fw.tile([128, 1], f32, tag="recip_s")
                    nc.vector.reduce_sum(
                        den_cs[0:HTS], dgT[0:HTS, :], axis=mybir.AxisListType.X
                    )
                    nc.vector.tensor_scalar_add(den_cs[0:HTS], den_cs[0:HTS], -PAD_ENT)
                    nc.vector.reciprocal(recip_s[0:HTS], den_cs[0:HTS])
                    # y rows for this slice
                    for n0 in range(0, D_EMB, 384):
                        po = fp.tile([128, 384], f32, tag="po")
                        for j in range(2):
                            nc.tensor.matmul(
                                po[:HTS, :],
                                pick_bf[:, j * TS + s * HTS : j * TS + (s + 1) * HTS],
                                wbt_sb[:, j * D_EMB + n0 : j * D_EMB + n0 + 384],
                                start=(j == 0),
                                stop=False,
                            )
                        nc.tensor.matmul(
                            po[:HTS, :],
                            onesrow[0:1, 0:HTS],
                            wbb_sb[0:1, n0 : n0 + 384],
                            start=False,
                            stop=True,
                        )
                        y_sb = fw.tile([128, 384], f32, tag="y_sb")
                        nc.vector.tensor_scalar_mul(
                            y_sb[:HTS, :], po[:HTS, :], recip_s[0:HTS, 0:1]
                        )
                        nc.sync.dma_start(
                            out=y_out[s * HTS : (s + 1) * HTS, n0 : n0 + 384],
                            in_=y_sb[:HTS, :],
                        )
                loss_sb = res.tile([1, 1], f32)
                nc.vector.reduce_sum(loss_sb, loss_acc, axis=mybir.AxisListType.X)
                nc.sync.dma_start(out=loss_out[:, :], in_=loss_sb)
            last_scatter = all_scatters[-1]
            for rd in post_reads:
                add_dep_helper(
                    rd.ins, last_scatter.ins, sync=False,
                    reason="post-A2A reads behind final scatter (queue HoL)",
                )

    nc.compile()
    return nc


def _chunked(x, kdim):
    """[kdim*128, F] -> [128, kdim*F] partition-major swizzle."""
    kd128, F = x.shape
    assert kd128 == kdim * 128
    return np.ascontiguousarray(
        x.reshape(kdim, 128, F).transpose(1, 0, 2).reshape(128, kdim * F)
    )


def _prep_host(X, bio, ents, Wf_w, Wf_b, E_w, Wb_w, Wb_b):
    inner = bio == INNER
    run = np.zeros((B, S), np.int64)
    run[:, S - 1] = inner[:, S - 1]
    for t in range(S - 2, -1, -1):
        run[:, t] = np.where(inner[:, t], run[:, t + 1] + 1, 0)
    run_next = np.concatenate([run[:, 1:], np.zeros((B, 1), np.int64)], axis=1)
    end_idx = np.arange(S, dtype=np.int64)[None, :] + run_next

    bb, ss = np.nonzero(bio == BEGIN)
    nb = bb.size
    T = max(128, ((nb + 127) // 128) * 128)
    TS = T // NCORES

    A = np.zeros((T, 2 * D_EMB), np.float32)
    if nb:
        A[:nb, :D_EMB] = X[bb, ss]
        A[:nb, D_EMB:] = X[bb, end_idx[bb, ss]]
    a_t = _chunked(np.ascontiguousarray(A.T), KA).astype(ml_dtypes.bfloat16)
    wf_t = _chunked(np.ascontiguousarray(Wf_w.T), KA).astype(ml_dtypes.bfloat16)
    wf_b = np.ascontiguousarray(Wf_b.reshape(2, 128).T).astype(np.float32)
    wb_t = _chunked(np.ascontiguousarray(Wb_w.T), 2).astype(ml_dtypes.bfloat16)
    wb_b = Wb_b.reshape(1, D_EMB).astype(ml_dtypes.bfloat16)

    ent_ids = ents[bb, ss] if nb else np.zeros((0,), np.int64)
    E_sel = np.zeros((D_ENT, T), np.float32)
    if nb:
        E_sel[:, :nb] = E_w[:, ent_ids]

    in_maps = []
    for c in range(NCORES):
        shard = np.zeros((D_ENT, NS), np.float32)
        shard[:, :SHARD] = E_w[:, c * SHARD : (c + 1) * SHARD]
        e_sh = _chunked(shard, 2).astype(ml_dtypes.bfloat16)
        e_sht = _chunked(np.ascontiguousarray(shard.T), NCH).astype(np.float16)
        esel_c = np.zeros((D_ENT, T), np.float32)
        lo, hi = c * TS, min((c + 1) * TS, nb)
        if hi > lo:
            esel_c[:, lo:hi] = E_sel[:, lo:hi]
        e_sel = _chunked(esel_c, 2).astype(ml_dtypes.bfloat16)
        m = np.zeros((1, TS), np.float32)
        if hi > lo:
            m[0, : hi - lo] = 1.0
        in_maps.append(
            {
                "a_t": a_t,
                "wf_t": wf_t,
                "wf_b": wf_b,
                "e_sh": e_sh,
                "e_sht": e_sht,
                "e_sel": e_sel,
                "mask": m,
                "wb_t": wb_t,
                "wb_b": wb_b,
            }
        )
    return in_maps, bb, ss, nb, T


def kernel(X, bio_output, entities_output, k, Wf_w, Wf_b, E_w, Wb_w, Wb_b):
    global LAST_RESULTS
    from concourse.bass_utils import run_bass_kernel_spmd

    X = np.asarray(X, np.float32)
    bio = np.asarray(bio_output).astype(np.int64)
    ents = np.asarray(entities_output).astype(np.int64)
    Wf_w = np.asarray(Wf_w, np.float32)
    Wf_b = np.asarray(Wf_b, np.float32)
    E_w = np.asarray(E_w, np.float32)
    Wb_w = np.asarray(Wb_w, np.float32)
    Wb_b = np.asarray(Wb_b, np.float32)

    in_maps, bb, ss, nb, T = _prep_host(X, bio, ents, Wf_w, Wf_b, E_w, Wb_w, Wb_b)

    if T not in _cache:
        _cache[T] = _build(T)
    nc = _cache[T]

    res = run_bass_kernel_spmd(
        nc, in_maps, core_ids=list(range(NCORES)), trace=TRACE
    )
    LAST_RESULTS = res

    y_full = np.concatenate([res.results[c]["y_out"] for c in range(NCORES)], axis=0)
    y = np.zeros((B, S, D_EMB), np.float32)
    if nb:
        y[bb, ss] = y_full[:nb]
    loss = np.zeros((1,), np.float32)
    loss[0] = sum(float(res.results[c]["loss_out"][0, 0]) for c in range(NCORES))
    return loss, y


# revision 17
# speedup vs baseline: 1.0660x; 1.0660x over previous
"""EntityMemory Trainium2 kernel (8 NeuronCores, entity-sharded).

Strategy:
  - Host: BIO span-end scan + compaction to BEGIN tokens only (~1/3 of B*S),
    builds span features A=[X[t], X[end(t)]] and per-core shards of the
    entity table E_w over N (model parallel, 6250 entities/core, padded 6272).
  - Device (SPMD, 8 cores): pseudo_T = Wf @ A_T; scores_T = E_shard.T @ pseudo_T
    per 128-entity chunk; exp on ACT (fp16); picked_T += E_shT.T @ exp (PSUM
    accum); softmax denominator accumulated on DVE (exp-sum) + one
    ones-matmul partition reduce; loss gather via host-gathered E columns
    (pseudo . E_sel) folded into the same ReduceScatter payload.
  - Tokens are processed in NSPLIT interleaved slices (each slice holds the
    same sub-range of every rank's tokens), and each slice gets its own
    bf16 ReduceScatter - earlier collectives overlap later compute.
  - Post-RS each core normalizes its token slice, applies Wb (+bias via a
    K=1 matmul row) and writes its y slice + loss partial.
  - Host: scatter y rows back to BEGIN positions, sum loss partials.

All device inputs are host-preswizzled to the exact [128, free] SBUF layout
so every input DMA is a contiguous partition-major copy.
"""

import sys

sys.path.insert(0, "/opt/trn_rl_repo")

import numpy as np
import ml_dtypes

B, S, D_EMB, D_ENT, N = 4, 512, 768, 256, 50000
BEGIN, INNER = 1, 2
NCORES = 8
SHARD = N // NCORES               # 6250 entities per core
NCH = (SHARD + 127) // 128        # 49 chunks
NS = NCH * 128                    # 6272 padded shard width
KA = (2 * D_EMB) // 128           # 12 contraction chunks for Wf
PAD_ENT = float(NCORES * NS - N)  # 176.0 pad entities, each contributing exp(0)=1

TRACE = False
LAST_RESULTS = None

_cache = {}


def _build(T):
    import concourse.mybir as mybir
    import concourse.tile as tile
    from concourse import bacc
    from concourse.tile_rust import add_dep_helper

    dt = mybir.dt
    f32, bf16, f16 = dt.float32, dt.bfloat16, dt.float16
    AF = mybir.ActivationFunctionType
    TS = T // NCORES
    MT = (TS + 127) // 128          # m-tiles over the final token slice
    NSPLIT = 2 if T <= 1024 else T // 512
    HTS = TS // NSPLIT              # per-rank tokens per slice
    NSL = NCORES * HTS              # matmul N per slice

    nc = bacc.Bacc("TRN2", target_bir_lowering=False, debug=False, num_devices=NCORES)

    a_t = nc.dram_tensor("a_t", [128, KA * T], bf16, kind="ExternalInput")
    wf_t = nc.dram_tensor("wf_t", [128, KA * D_ENT], bf16, kind="ExternalInput")
    wf_b = nc.dram_tensor("wf_b", [128, 2], f32, kind="ExternalInput")
    e_sh = nc.dram_tensor("e_sh", [128, 2 * NS], bf16, kind="ExternalInput")
    e_sht = nc.dram_tensor("e_sht", [128, NCH * D_ENT], f16, kind="ExternalInput")
    e_sel = nc.dram_tensor("e_sel", [128, 2 * T], bf16, kind="ExternalInput")
    mask = nc.dram_tensor("mask", [1, TS], f32, kind="ExternalInput")
    wb_t = nc.dram_tensor("wb_t", [128, 2 * D_EMB], bf16, kind="ExternalInput")
    wb_b = nc.dram_tensor("wb_b", [1, D_EMB], bf16, kind="ExternalInput")
    y_out = nc.dram_tensor("y_out", [TS, D_EMB], f32, kind="ExternalOutput")
    loss_out = nc.dram_tensor("loss_out", [1, 1], f32, kind="ExternalOutput")

    def r3(ap, j, s):
        """Interleaved-slice view: [128, 8, HTS] of column block j, slice s."""
        return ap[:, j * T : (j + 1) * T].rearrange("p (r w) -> p r w", r=NCORES)[
            :, :, s * HTS : (s + 1) * HTS
        ]

    with tile.TileContext(nc) as tc:
        with (
            tc.tile_pool(name="res", bufs=1) as res,
            tc.tile_pool(name="work", bufs=3) as work,
            tc.tile_pool(name="exl", bufs=8) as exl,
            tc.tile_pool(name="dram", bufs=1, space="DRAM") as dram,
        ):
            # ---- resident SBUF tiles (contiguous chunked DMAs) ----
            a_sb = res.tile([128, KA * T], bf16)
            wf_sb = res.tile([128, KA * D_ENT], bf16)
            wfb_sb = res.tile([128, 2], f32)
            esh_sb = res.tile([128, 2 * NS], bf16)
            esht_sb = res.tile([128, NCH * D_ENT], f16)
            esel_sb = res.tile([128, 2 * T], bf16)
            mask_sb = res.tile([1, TS], f32)
            wbt_sb = res.tile([128, 2 * D_EMB], bf16)
            wbb_sb = res.tile([1, D_EMB], bf16)
            ones32 = res.tile([128, 1], f32)
            onesrow = res.tile([1, TS], bf16)
            pseudo_sb = res.tile([128, 2 * T], bf16)
            prod_sb = res.tile([128, 2 * T], f32)
            acc_sb = res.tile([128, T], f32)

            nc.vector.memset(ones32, 1.0)
            nc.vector.memset(onesrow, 1.0)
            nc.vector.memset(acc_sb, 0.0)

            # pseudo-phase inputs first, interleaved so matmul k can start as
            # soon as chunk k lands
            for kk in range(KA):
                nc.sync.dma_start(
                    out=a_sb[:, kk * T : (kk + 1) * T],
                    in_=a_t[:, kk * T : (kk + 1) * T],
                )
                nc.sync.dma_start(
                    out=wf_sb[:, kk * D_ENT : (kk + 1) * D_ENT],
                    in_=wf_t[:, kk * D_ENT : (kk + 1) * D_ENT],
                )
            nc.sync.dma_start(out=wfb_sb, in_=wf_b[:, :])
            EGRP = 7
            for c0 in range(0, NCH, EGRP):
                w = min(EGRP, NCH - c0) * 128
                for j in range(2):
                    nc.sync.dma_start(
                        out=esh_sb[:, j * NS + c0 * 128 : j * NS + c0 * 128 + w],
                        in_=e_sh[:, j * NS + c0 * 128 : j * NS + c0 * 128 + w],
                    )
                w = min(EGRP, NCH - c0) * D_ENT
                nc.sync.dma_start(
                    out=esht_sb[:, c0 * D_ENT : c0 * D_ENT + w],
                    in_=e_sht[:, c0 * D_ENT : c0 * D_ENT + w],
                )
            nc.sync.dma_start(out=esel_sb, in_=e_sel[:, :])
            nc.sync.dma_start(out=mask_sb, in_=mask[:, :])
            nc.sync.dma_start(out=wbt_sb, in_=wb_t[:, :])
            nc.sync.dma_start(out=wbb_sb, in_=wb_b[:, :])

            rs_ins = [dram.tile([NCORES, 258 * HTS], bf16, tag=f"rsi{s}", name=f"rs_in{s}") for s in range(NSPLIT)]
            rs_outs = [dram.tile([NCORES, 258 * HTS], bf16, tag=f"rso{s}", name=f"rs_out{s}") for s in range(NSPLIT)]

            # ---- phase 1: pseudo_T[d_ent, tok] = Wf @ A_T + Wf_b ----
            # computed per interleaved slice so the entity loop starts sooner;
            # prod = pseudo * E_sel feeds the loss-gather row
            with tc.tile_pool(name="pp", bufs=2, space="PSUM") as pp:
                for s in range(NSPLIT):
                    for j in range(2):
                        ps = pp.tile([128, 512], f32)
                        for kk in range(KA):
                            nc.tensor.matmul(
                                ps[:, :NSL],
                                wf_sb[:, kk * D_ENT + j * 128 : kk * D_ENT + (j + 1) * 128],
                                a_sb[:, kk * T : (kk + 1) * T].rearrange(
                                    "p (r w) -> p r w", r=NCORES
                                )[:, :, s * HTS : (s + 1) * HTS],
                                start=(kk == 0),
                                stop=(kk == KA - 1),
                            )
                        nc.vector.tensor_scalar_add(
                            r3(pseudo_sb, j, s),
                            ps[:, :NSL].rearrange("p (r w) -> p r w", r=NCORES),
                            wfb_sb[:, j : j + 1],
                        )
                        nc.vector.tensor_mul(
                            r3(prod_sb, j, s),
                            r3(pseudo_sb, j, s),
                            r3(esel_sb, j, s),
                        )

            # ---- phase 2: entity-chunk loop per interleaved slice ----
            with (
                tc.tile_pool(name="sp", bufs=4, space="PSUM") as sp,
                tc.tile_pool(name="ac", bufs=1, space="PSUM") as ac,
            ):
                prev_coll = None
                all_scatters = []
                for s in range(NSPLIT):
                    pk0 = ac.tile([128, 512], f32, tag="pk0")
                    pk1 = ac.tile([128, 512], f32, tag="pk1")
                    pks = [pk0, pk1]
                    for c in range(NCH):
                        sc = sp.tile([128, 512], f32)
                        for j in range(2):
                            nc.tensor.matmul(
                                sc[:, :NSL],
                                esh_sb[:, j * NS + c * 128 : j * NS + (c + 1) * 128],
                                r3(pseudo_sb, j, s),
                                start=(j == 0),
                                stop=(j == 1),
                            )
                        ex = exl.tile([128, 512], f16, tag="ex")
                        nc.scalar.activation(ex[:, :NSL], sc[:, :NSL], AF.Exp)
                        for m in range(2):
                            nc.tensor.matmul(
                                pks[m][:, :NSL],
                                esht_sb[:, c * D_ENT + m * 128 : c * D_ENT + (m + 1) * 128],
                                ex[:, :NSL],
                                start=(c == 0),
                                stop=(c == NCH - 1),
                            )
                        # denominator partial on DVE (keeps PE free)
                        nc.vector.tensor_add(
                            r3(acc_sb, 0, s),
                            r3(acc_sb, 0, s),
                            ex[:, :NSL].rearrange("p (r w) -> p r w", r=NCORES),
                        )
                    # partition-reduce of exp-sum + gather row for this slice
                    den = ac.tile([1, 512], f32, tag="den")
                    gps = ac.tile([1, 512], f32, tag="gps")
                    nc.tensor.matmul(
                        den[:1, :NSL], ones32[:, 0:1], r3(acc_sb, 0, s),
                        start=True, stop=True,
                    )
                    for j in range(2):
                        nc.tensor.matmul(
                            gps[:1, :NSL], ones32[:, 0:1], r3(prod_sb, j, s),
                            start=(j == 0), stop=(j == 1),
                        )
                    # copy partials out and scatter into RS payload blocks.
                    # dst view [d, r, w]: block r at r*258*HTS, row d at d*HTS.
                    dst3 = rs_ins[s][:, :].rearrange("r (d w) -> d r w", d=258)
                    scatter_dmas = []
                    for m in range(2):
                        pk_sb = work.tile([128, 512], bf16, tag="pk_sb")
                        nc.vector.tensor_copy(pk_sb[:, :NSL], pks[m][:, :NSL])
                        scatter_dmas.append(nc.sync.dma_start(
                            out=dst3[m * 128 : (m + 1) * 128, :, :],
                            in_=pk_sb[:, :NSL].rearrange("p (r w) -> p r w", r=NCORES),
                        ))
                    den_sb = work.tile([1, 512], bf16, tag="den_sb")
                    g_sb = work.tile([1, 512], bf16, tag="g_sb")
                    nc.vector.tensor_copy(den_sb[:1, :NSL], den[:1, :NSL])
                    nc.vector.tensor_copy(g_sb[:1, :NSL], gps[:1, :NSL])
                    scatter_dmas.append(nc.sync.dma_start(
                        out=dst3[256:257, :, :],
                        in_=den_sb[:1, :NSL].rearrange("p (r w) -> p r w", r=NCORES),
                    ))
                    scatter_dmas.append(nc.sync.dma_start(
                        out=dst3[257:258, :, :],
                        in_=g_sb[:1, :NSL].rearrange("p (r w) -> p r w", r=NCORES),
                    ))
                    # keep later slices' scatter DMAs ordered after this
                    # slice's collective so its queue-sem waits stay tight
                    all_scatters.extend(scatter_dmas)
                    if prev_coll is not None:
                        for dma_i in scatter_dmas:
                            add_dep_helper(
                                dma_i.ins, prev_coll.ins, sync=False,
                                reason="scatter after prior slice RS",
                            )
                    # slice-s ReduceScatter: overlaps the next slice's compute
                    coll = nc.gpsimd.collective_compute(
                        "AllToAll",
                        mybir.AluOpType.bypass,
                        replica_groups=[list(range(NCORES))],
                        ins=[rs_ins[s][:, :].opt()],
                        outs=[rs_outs[s][:, :].opt()],
                    )
                    prev_coll = coll

            # ---- phase 4 (per slice): local 8-way reduce of A2A blocks,
            # normalize, down-project, write y slice + loss partial ----
            ones_bf = res.tile([128, 1], bf16)
            nc.vector.memset(ones_bf, 1.0)
            pick_bf = res.tile([128, 2 * TS], bf16)
            den_rf = res.tile([1, TS], f32)
            recip_row = res.tile([1, TS], f32)
            loss_acc = res.tile([1, NSPLIT], f32)
            post_reads = []
            with (
                tc.tile_pool(name="fp", bufs=2, space="PSUM") as fp,
                tc.tile_pool(name="fw", bufs=2) as fw,
            ):
                for s in range(NSPLIT):
                    # src view [d, i, w]: contributor i's row d for my tokens
                    src3 = rs_outs[s][:, :].rearrange("i (d w) -> d i w", d=258)
                    for m in range(2):
                        blk = fw.tile([128, NCORES * HTS], bf16, tag="blk")
                        post_reads.append(nc.sync.dma_start(
                            out=blk.rearrange("p (i w) -> p i w", i=NCORES),
                            in_=src3[m * 128 : (m + 1) * 128, :, :],
                        ))
                        pacc = fw.tile([128, HTS], f32, tag="pacc")
                        nc.vector.tensor_add(
                            pacc, blk[:, 0:HTS], blk[:, HTS : 2 * HTS]
                        )
                        for i in range(2, NCORES):
                            nc.vector.tensor_add(
                                pacc, pacc, blk[:, i * HTS : (i + 1) * HTS]
                            )
                        nc.vector.tensor_copy(
                            pick_bf[:, m * TS + s * HTS : m * TS + (s + 1) * HTS], pacc
                        )
                    dg = fw.tile([8, 2 * HTS], bf16, tag="dg")
                    post_reads.append(nc.sync.dma_start(
                        out=dg[:, 0:HTS], in_=rs_outs[s][:, 256 * HTS : 257 * HTS]
                    ))
                    post_reads.append(nc.sync.dma_start(
                        out=dg[:, HTS : 2 * HTS], in_=rs_outs[s][:, 257 * HTS : 258 * HTS]
                    ))
                    den_ps = fp.tile([1, 512], f32, tag="den_ps")
                    g_ps = fp.tile([1, 512], f32, tag="g_ps")
                    nc.tensor.matmul(
                        den_ps[0:1, :HTS], ones_bf[0:8, 0:1], dg[:, 0:HTS],
                        start=True, stop=True,
                    )
                    nc.tensor.matmul(
                        g_ps[0:1, :HTS], ones_bf[0:8, 0:1], dg[:, HTS : 2 * HTS],
                        start=True, stop=True,
                    )
                    # denominator (minus zero-pad entity contribution) + recip
                    nc.vector.tensor_scalar_add(
                        den_rf[:, s * HTS : (s + 1) * HTS], den_ps[0:1, :HTS], -PAD_ENT
                    )
                    nc.vector.reciprocal(
                        recip_row[:, s * HTS : (s + 1) * HTS],
                        den_rf[:, s * HTS : (s + 1) * HTS],
                    )
                    # loss partial: sum(mask * exp(g) / den) over this slice
                    g_exp = fw.tile([1, HTS], f32, tag="g_exp")
                    nc.scalar.activation(g_exp, g_ps[0:1, :HTS], AF.Exp)
                    nc.vector.tensor_mul(
                        g_exp, g_exp, recip_row[:, s * HTS : (s + 1) * HTS]
                    )
                    nc.vector.tensor_mul(
                        g_exp, g_exp, mask_sb[:, s * HTS : (s + 1) * HTS]
                    )
                    nc.vector.reduce_sum(
                        loss_acc[:, s : s + 1], g_exp, axis=mybir.AxisListType.X
                    )
                    # per-token reciprocal column: transposed view [w, i] of
                    # the 8 contributors' den rows, reduced along free dim
                    dgT = fw.tile([128, NCORES], bf16, tag="dgT")
                    post_reads.append(nc.sync.dma_start(
                        out=dgT[0:HTS, :],
                        in_=rs_outs[s][:, 256 * HTS : 257 * HTS].rearrange(
                            "i w -> w i"
                        ),
                    ))
                    den_cs = fw.tile([128, 1], f32, tag="den_cs")
                    recip_s = fw.tile([128, 1], f32, tag="recip_s")
                    nc.vector.reduce_sum(
                        den_cs[0:HTS], dgT[0:HTS, :], axis=mybir.AxisListType.X
                    )
                    nc.vector.tensor_scalar_add(den_cs[0:HTS], den_cs[0:HTS], -PAD_ENT)
                    nc.vector.reciprocal(recip_s[0:HTS], den_cs[0:HTS])
                    # y rows for this slice
                    for n0 in range(0, D_EMB, 384):
                        po = fp.tile([128, 384], f32, tag="po")
                        for j in range(2):
                            nc.tensor.matmul(
                                po[:HTS, :],
                                pick_bf[:, j * TS + s * HTS : j * TS + (s + 1) * HTS],
                                wbt_sb[:, j * D_EMB + n0 : j * D_EMB + n0 + 384],
                                start=(j == 0),
                                stop=False,
                            )
                        nc.tensor.matmul(
                            po[:HTS, :],
                            onesrow[0:1, 0:HTS],
                            wbb_sb[0:1, n0 : n0 + 384],
                            start=False,
                            stop=True,
                        )
                        y_sb = fw.tile([128, 384], f32, tag="y_sb")
                        nc.vector.tensor_scalar_mul(
                            y_sb[:HTS, :], po[:HTS, :], recip_s[0:HTS, 0:1]
                        )
                        nc.sync.dma_start(
                            out=y_out[s * HTS : (s + 1) * HTS, n0 : n0 + 384],
                            in_=y_sb[:HTS, :],
                        )
                loss_sb = res.tile([1, 1], f32)
                nc.vector.reduce_sum(loss_sb, loss_acc, axis=mybir.AxisListType.X)
                nc.sync.dma_start(out=loss_out[:, :], in_=loss_sb)
            last_scatter = all_scatters[-1]
            for rd in post_reads:
                add_dep_helper(
                    rd.ins, last_scatter.ins, sync=False,
                    reason="post-A2A reads behind final scatter (queue HoL)",
                )

    nc.compile()
    return nc


def _chunked(x, kdim):
    """[kdim*128, F] -> [128, kdim*F] partition-major swizzle."""
    kd128, F = x.shape
    assert kd128 == kdim * 128
    return np.ascontiguousarray(
        x.reshape(kdim, 128, F).transpose(1, 0, 2).reshape(128, kdim * F)
    )


def _prep_host(X, bio, ents, Wf_w, Wf_b, E_w, Wb_w, Wb_b):
    inner = bio == INNER
    run = np.zeros((B, S), np.int64)
    run[:, S - 1] = inner[:, S - 1]
    for t in range(S - 2, -1, -1):
        run[:, t] = np.where(inner[:, t], run[:, t + 1] + 1, 0)
    run_next = np.concatenate([run[:, 1:], np.zeros((B, 1), np.int64)], axis=1)
    end_idx = np.arange(S, dtype=np.int64)[None, :] + run_next

    bb, ss = np.nonzero(bio == BEGIN)
    nb = bb.size
    T = max(128, ((nb + 127) // 128) * 128)
    TS = T // NCORES

    A = np.zeros((T, 2 * D_EMB), np.float32)
    if nb:
        A[:nb, :D_EMB] = X[bb, ss]
        A[:nb, D_EMB:] = X[bb, end_idx[bb, ss]]
    a_t = _chunked(np.ascontiguousarray(A.T), KA).astype(ml_dtypes.bfloat16)
    wf_t = _chunked(np.ascontiguousarray(Wf_w.T), KA).astype(ml_dtypes.bfloat16)
    wf_b = np.ascontiguousarray(Wf_b.reshape(2, 128).T).astype(np.float32)
    wb_t = _chunked(np.ascontiguousarray(Wb_w.T), 2).astype(ml_dtypes.bfloat16)
    wb_b = Wb_b.reshape(1, D_EMB).astype(ml_dtypes.bfloat16)

    ent_ids = ents[bb, ss] if nb else np.zeros((0,), np.int64)
    E_sel = np.zeros((D_ENT, T), np.float32)
    if nb:
        E_sel[:, :nb] = E_w[:, ent_ids]

    in_maps = []
    for c in range(NCORES):
        shard = np.zeros((D_ENT, NS), np.float32)
        shard[:, :SHARD] = E_w[:, c * SHARD : (c + 1) * SHARD]
        e_sh = _chunked(shard, 2).astype(ml_dtypes.bfloat16)
        e_sht = _chunked(np.ascontiguousarray(shard.T), NCH).astype(np.float16)
        esel_c = np.zeros((D_ENT, T), np.float32)
        lo, hi = c * TS, min((c + 1) * TS, nb)
        if hi > lo:
            esel_c[:, lo:hi] = E_sel[:, lo:hi]
        e_sel = _chunked(esel_c, 2).astype(ml_dtypes.bfloat16)
        m = np.zeros((1, TS), np.float32)
        if hi > lo:
            m[0, : hi - lo] = 1.0
        in_maps.append(
            {
                "a_t": a_t,
                "wf_t": wf_t,
                "wf_b": wf_b,
                "e_sh": e_sh,
                "e_sht": e_sht,
                "e_sel": e_sel,
                "mask": m,
                "wb_t": wb_t,
                "wb_b": wb_b,
            }
        )
    return in_maps, bb, ss, nb, T


def kernel(X, bio_output, entities_output, k, Wf_w, Wf_b, E_w, Wb_w, Wb_b):
    global LAST_RESULTS
    from concourse.bass_utils import run_bass_kernel_spmd

    X = np.asarray(X, np.float32)
    bio = np.asarray(bio_output).astype(np.int64)
    ents = np.asarray(entities_output).astype(np.int64)
    Wf_w = np.asarray(Wf_w, np.float32)
    Wf_b = np.asarray(Wf_b, np.float32)
    E_w = np.asarray(E_w, np.float32)
    Wb_w = np.asarray(Wb_w, np.float32)
    Wb_b = np.asarray(Wb_b, np.float32)

    in_maps, bb, ss, nb, T = _prep_host(X, bio, ents, Wf_w, Wf_b, E_w, Wb_w, Wb_b)

    if T not in _cache:
        _cache[T] = _build(T)
    nc = _cache[T]

    res = run_bass_kernel_spmd(
        nc, in_maps, core_ids=list(range(NCORES)), trace=TRACE
    )
    LAST_RESULTS = res

    y_full = np.concatenate([res.results[c]["y_out"] for c in range(NCORES)], axis=0)
    y = np.zeros((B, S, D_EMB), np.float32)
    if nb:
        y[bb, ss] = y_full[:nb]
    loss = np.zeros((1,), np.float32)
    loss[0] = sum(float(res.results[c]["loss_out"][0, 0]) for c in range(NCORES))
    return loss, y


# revision 18
# speedup vs baseline: 1.0792x; 1.0124x over previous
"""EntityMemory Trainium2 kernel (8 NeuronCores, entity-sharded).

Strategy:
  - Host: BIO span-end scan + compaction to BEGIN tokens only (~1/3 of B*S),
    builds span features A=[X[t], X[end(t)]] and per-core shards of the
    entity table E_w over N (model parallel, 6250 entities/core, padded 6272).
  - Device (SPMD, 8 cores): pseudo_T = Wf @ A_T; scores_T = E_shard.T @ pseudo_T
    per 128-entity chunk; exp on ACT (fp16); picked_T += E_shT.T @ exp (PSUM
    accum); softmax denominator accumulated on DVE (exp-sum) + one
    ones-matmul partition reduce; loss gather via host-gathered E columns
    (pseudo . E_sel) folded into the same ReduceScatter payload.
  - Tokens are processed in NSPLIT interleaved slices (each slice holds the
    same sub-range of every rank's tokens), and each slice gets its own
    bf16 ReduceScatter - earlier collectives overlap later compute.
  - Post-RS each core normalizes its token slice, applies Wb (+bias via a
    K=1 matmul row) and writes its y slice + loss partial.
  - Host: scatter y rows back to BEGIN positions, sum loss partials.

All device inputs are host-preswizzled to the exact [128, free] SBUF layout
so every input DMA is a contiguous partition-major copy.
"""

import sys

sys.path.insert(0, "/opt/trn_rl_repo")

import numpy as np
import ml_dtypes

B, S, D_EMB, D_ENT, N = 4, 512, 768, 256, 50000
BEGIN, INNER = 1, 2
NCORES = 8
SHARD = N // NCORES               # 6250 entities per core
NCH = (SHARD + 127) // 128        # 49 chunks
NS = NCH * 128                    # 6272 padded shard width
KA = (2 * D_EMB) // 128           # 12 contraction chunks for Wf
PAD_ENT = float(NCORES * NS - N)  # 176.0 pad entities, each contributing exp(0)=1

TRACE = False
LAST_RESULTS = None

_cache = {}


def _build(T):
    import concourse.mybir as mybir
    import concourse.tile as tile
    from concourse import bacc
    from concourse.tile_rust import add_dep_helper

    dt = mybir.dt
    f32, bf16, f16 = dt.float32, dt.bfloat16, dt.float16
    AF = mybir.ActivationFunctionType
    TS = T // NCORES
    MT = (TS + 127) // 128          # m-tiles over the final token slice
    NSPLIT = 3 if T <= 1024 else T // 512
    HTS = TS // NSPLIT              # per-rank tokens per slice
    NSL = NCORES * HTS              # matmul N per slice

    nc = bacc.Bacc("TRN2", target_bir_lowering=False, debug=False, num_devices=NCORES)

    a_t = nc.dram_tensor("a_t", [128, KA * T], bf16, kind="ExternalInput")
    wf_t = nc.dram_tensor("wf_t", [128, KA * D_ENT], bf16, kind="ExternalInput")
    wf_b = nc.dram_tensor("wf_b", [128, 2], f32, kind="ExternalInput")
    e_sh = nc.dram_tensor("e_sh", [128, 2 * NS], bf16, kind="ExternalInput")
    e_sht = nc.dram_tensor("e_sht", [128, NCH * D_ENT], f16, kind="ExternalInput")
    e_sel = nc.dram_tensor("e_sel", [128, 2 * T], bf16, kind="ExternalInput")
    mask = nc.dram_tensor("mask", [1, TS], f32, kind="ExternalInput")
    wb_t = nc.dram_tensor("wb_t", [128, 2 * D_EMB], bf16, kind="ExternalInput")
    wb_b = nc.dram_tensor("wb_b", [1, D_EMB], bf16, kind="ExternalInput")
    y_out = nc.dram_tensor("y_out", [TS, D_EMB], f32, kind="ExternalOutput")
    loss_out = nc.dram_tensor("loss_out", [1, 1], f32, kind="ExternalOutput")

    def r3(ap, j, s):
        """Interleaved-slice view: [128, 8, HTS] of column block j, slice s."""
        return ap[:, j * T : (j + 1) * T].rearrange("p (r w) -> p r w", r=NCORES)[
            :, :, s * HTS : (s + 1) * HTS
        ]

    with tile.TileContext(nc) as tc:
        with (
            tc.tile_pool(name="res", bufs=1) as res,
            tc.tile_pool(name="work", bufs=3) as work,
            tc.tile_pool(name="exl", bufs=8) as exl,
            tc.tile_pool(name="dram", bufs=1, space="DRAM") as dram,
        ):
            # ---- resident SBUF tiles (contiguous chunked DMAs) ----
            a_sb = res.tile([128, KA * T], bf16)
            wf_sb = res.tile([128, KA * D_ENT], bf16)
            wfb_sb = res.tile([128, 2], f32)
            esh_sb = res.tile([128, 2 * NS], bf16)
            esht_sb = res.tile([128, NCH * D_ENT], f16)
            esel_sb = res.tile([128, 2 * T], bf16)
            mask_sb = res.tile([1, TS], f32)
            wbt_sb = res.tile([128, 2 * D_EMB], bf16)
            wbb_sb = res.tile([1, D_EMB], bf16)
            ones32 = res.tile([128, 1], f32)
            onesrow = res.tile([1, TS], bf16)
            pseudo_sb = res.tile([128, 2 * T], bf16)
            prod_sb = res.tile([128, 2 * T], f32)
            acc_sb = res.tile([128, T], f32)

            nc.vector.memset(ones32, 1.0)
            nc.vector.memset(onesrow, 1.0)
            nc.vector.memset(acc_sb, 0.0)

            # pseudo-phase inputs first, interleaved so matmul k can start as
            # soon as chunk k lands
            for kk in range(KA):
                nc.sync.dma_start(
                    out=a_sb[:, kk * T : (kk + 1) * T],
                    in_=a_t[:, kk * T : (kk + 1) * T],
                )
                nc.sync.dma_start(
                    out=wf_sb[:, kk * D_ENT : (kk + 1) * D_ENT],
                    in_=wf_t[:, kk * D_ENT : (kk + 1) * D_ENT],
                )
            nc.sync.dma_start(out=wfb_sb, in_=wf_b[:, :])
            EGRP = 7
            for c0 in range(0, NCH, EGRP):
                w = min(EGRP, NCH - c0) * 128
                for j in range(2):
                    nc.sync.dma_start(
                        out=esh_sb[:, j * NS + c0 * 128 : j * NS + c0 * 128 + w],
                        in_=e_sh[:, j * NS + c0 * 128 : j * NS + c0 * 128 + w],
                    )
                w = min(EGRP, NCH - c0) * D_ENT
                nc.sync.dma_start(
                    out=esht_sb[:, c0 * D_ENT : c0 * D_ENT + w],
                    in_=e_sht[:, c0 * D_ENT : c0 * D_ENT + w],
                )
            nc.sync.dma_start(out=esel_sb, in_=e_sel[:, :])
            nc.sync.dma_start(out=mask_sb, in_=mask[:, :])
            nc.sync.dma_start(out=wbt_sb, in_=wb_t[:, :])
            nc.sync.dma_start(out=wbb_sb, in_=wb_b[:, :])

            rs_ins = [dram.tile([NCORES, 258 * HTS], bf16, tag=f"rsi{s}", name=f"rs_in{s}") for s in range(NSPLIT)]
            rs_outs = [dram.tile([NCORES, 258 * HTS], bf16, tag=f"rso{s}", name=f"rs_out{s}") for s in range(NSPLIT)]

            # ---- phase 1: pseudo_T[d_ent, tok] = Wf @ A_T + Wf_b ----
            # computed per interleaved slice so the entity loop starts sooner;
            # prod = pseudo * E_sel feeds the loss-gather row
            with tc.tile_pool(name="pp", bufs=2, space="PSUM") as pp:
                for s in range(NSPLIT):
                    for j in range(2):
                        ps = pp.tile([128, 512], f32)
                        for kk in range(KA):
                            nc.tensor.matmul(
                                ps[:, :NSL],
                                wf_sb[:, kk * D_ENT + j * 128 : kk * D_ENT + (j + 1) * 128],
                                a_sb[:, kk * T : (kk + 1) * T].rearrange(
                                    "p (r w) -> p r w", r=NCORES
                                )[:, :, s * HTS : (s + 1) * HTS],
                                start=(kk == 0),
                                stop=(kk == KA - 1),
                            )
                        nc.vector.tensor_scalar_add(
                            r3(pseudo_sb, j, s),
                            ps[:, :NSL].rearrange("p (r w) -> p r w", r=NCORES),
                            wfb_sb[:, j : j + 1],
                        )
                        nc.vector.tensor_mul(
                            r3(prod_sb, j, s),
                            r3(pseudo_sb, j, s),
                            r3(esel_sb, j, s),
                        )

            # ---- phase 2: entity-chunk loop per interleaved slice ----
            with (
                tc.tile_pool(name="sp", bufs=4, space="PSUM") as sp,
                tc.tile_pool(name="ac", bufs=1, space="PSUM") as ac,
            ):
                prev_coll = None
                all_scatters = []
                for s in range(NSPLIT):
                    pk0 = ac.tile([128, 512], f32, tag="pk0")
                    pk1 = ac.tile([128, 512], f32, tag="pk1")
                    pks = [pk0, pk1]
                    for c in range(NCH):
                        sc = sp.tile([128, 512], f32)
                        for j in range(2):
                            nc.tensor.matmul(
                                sc[:, :NSL],
                                esh_sb[:, j * NS + c * 128 : j * NS + (c + 1) * 128],
                                r3(pseudo_sb, j, s),
                                start=(j == 0),
                                stop=(j == 1),
                            )
                        ex = exl.tile([128, 512], f16, tag="ex")
                        nc.scalar.activation(ex[:, :NSL], sc[:, :NSL], AF.Exp)
                        for m in range(2):
                            nc.tensor.matmul(
                                pks[m][:, :NSL],
                                esht_sb[:, c * D_ENT + m * 128 : c * D_ENT + (m + 1) * 128],
                                ex[:, :NSL],
                                start=(c == 0),
                                stop=(c == NCH - 1),
                            )
                        # denominator partial on DVE (keeps PE free)
                        nc.vector.tensor_add(
                            r3(acc_sb, 0, s),
                            r3(acc_sb, 0, s),
                            ex[:, :NSL].rearrange("p (r w) -> p r w", r=NCORES),
                        )
                    # partition-reduce of exp-sum + gather row for this slice
                    den = ac.tile([1, 512], f32, tag="den")
                    gps = ac.tile([1, 512], f32, tag="gps")
                    nc.tensor.matmul(
                        den[:1, :NSL], ones32[:, 0:1], r3(acc_sb, 0, s),
                        start=True, stop=True,
                    )
                    for j in range(2):
                        nc.tensor.matmul(
                            gps[:1, :NSL], ones32[:, 0:1], r3(prod_sb, j, s),
                            start=(j == 0), stop=(j == 1),
                        )
                    # copy partials out and scatter into RS payload blocks.
                    # dst view [d, r, w]: block r at r*258*HTS, row d at d*HTS.
                    dst3 = rs_ins[s][:, :].rearrange("r (d w) -> d r w", d=258)
                    scatter_dmas = []
                    for m in range(2):
                        pk_sb = work.tile([128, 512], bf16, tag="pk_sb")
                        nc.vector.tensor_copy(pk_sb[:, :NSL], pks[m][:, :NSL])
                        scatter_dmas.append(nc.sync.dma_start(
                            out=dst3[m * 128 : (m + 1) * 128, :, :],
                            in_=pk_sb[:, :NSL].rearrange("p (r w) -> p r w", r=NCORES),
                        ))
                    den_sb = work.tile([1, 512], bf16, tag="den_sb")
                    g_sb = work.tile([1, 512], bf16, tag="g_sb")
                    nc.vector.tensor_copy(den_sb[:1, :NSL], den[:1, :NSL])
                    nc.vector.tensor_copy(g_sb[:1, :NSL], gps[:1, :NSL])
                    scatter_dmas.append(nc.sync.dma_start(
                        out=dst3[256:257, :, :],
                        in_=den_sb[:1, :NSL].rearrange("p (r w) -> p r w", r=NCORES),
                    ))
                    scatter_dmas.append(nc.sync.dma_start(
                        out=dst3[257:258, :, :],
                        in_=g_sb[:1, :NSL].rearrange("p (r w) -> p r w", r=NCORES),
                    ))
                    # keep later slices' scatter DMAs ordered after this
                    # slice's collective so its queue-sem waits stay tight
                    all_scatters.extend(scatter_dmas)
                    if prev_coll is not None:
                        for dma_i in scatter_dmas:
                            add_dep_helper(
                                dma_i.ins, prev_coll.ins, sync=False,
                                reason="scatter after prior slice RS",
                            )
                    # slice-s ReduceScatter: overlaps the next slice's compute
                    coll = nc.gpsimd.collective_compute(
                        "AllToAll",
                        mybir.AluOpType.bypass,
                        replica_groups=[list(range(NCORES))],
                        ins=[rs_ins[s][:, :].opt()],
                        outs=[rs_outs[s][:, :].opt()],
                    )
                    prev_coll = coll

            # ---- phase 4 (per slice): local 8-way reduce of A2A blocks,
            # normalize, down-project, write y slice + loss partial ----
            ones_bf = res.tile([128, 1], bf16)
            nc.vector.memset(ones_bf, 1.0)
            pick_bf = res.tile([128, 2 * TS], bf16)
            den_rf = res.tile([1, TS], f32)
            recip_row = res.tile([1, TS], f32)
            loss_acc = res.tile([1, NSPLIT], f32)
            post_reads = []
            with (
                tc.tile_pool(name="fp", bufs=2, space="PSUM") as fp,
                tc.tile_pool(name="fw", bufs=2) as fw,
            ):
                for s in range(NSPLIT):
                    # src view [d, i, w]: contributor i's row d for my tokens
                    src3 = rs_outs[s][:, :].rearrange("i (d w) -> d i w", d=258)
                    for m in range(2):
                        blk = fw.tile([128, NCORES * HTS], bf16, tag="blk")
                        post_reads.append(nc.sync.dma_start(
                            out=blk.rearrange("p (i w) -> p i w", i=NCORES),
                            in_=src3[m * 128 : (m + 1) * 128, :, :],
                        ))
                        pacc = fw.tile([128, HTS], f32, tag="pacc")
                        nc.vector.tensor_add(
                            pacc, blk[:, 0:HTS], blk[:, HTS : 2 * HTS]
                        )
                        for i in range(2, NCORES):
                            nc.vector.tensor_add(
                                pacc, pacc, blk[:, i * HTS : (i + 1) * HTS]
                            )
                        nc.vector.tensor_copy(
                            pick_bf[:, m * TS + s * HTS : m * TS + (s + 1) * HTS], pacc
                        )
                    dg = fw.tile([8, 2 * HTS], bf16, tag="dg")
                    post_reads.append(nc.sync.dma_start(
                        out=dg[:, 0:HTS], in_=rs_outs[s][:, 256 * HTS : 257 * HTS]
                    ))
                    post_reads.append(nc.sync.dma_start(
                        out=dg[:, HTS : 2 * HTS], in_=rs_outs[s][:, 257 * HTS : 258 * HTS]
                    ))
                    den_ps = fp.tile([1, 512], f32, tag="den_ps")
                    g_ps = fp.tile([1, 512], f32, tag="g_ps")
                    nc.tensor.matmul(
                        den_ps[0:1, :HTS], ones_bf[0:8, 0:1], dg[:, 0:HTS],
                        start=True, stop=True,
                    )
                    nc.tensor.matmul(
                        g_ps[0:1, :HTS], ones_bf[0:8, 0:1], dg[:, HTS : 2 * HTS],
                        start=True, stop=True,
                    )
                    # denominator (minus zero-pad entity contribution) + recip
                    nc.vector.tensor_scalar_add(
                        den_rf[:, s * HTS : (s + 1) * HTS], den_ps[0:1, :HTS], -PAD_ENT
                    )
                    nc.vector.reciprocal(
                        recip_row[:, s * HTS : (s + 1) * HTS],
                        den_rf[:, s * HTS : (s + 1) * HTS],
                    )
                    # loss partial: sum(mask * exp(g) / den) over this slice
                    g_exp = fw.tile([1, HTS], f32, tag="g_exp")
                    nc.scalar.activation(g_exp, g_ps[0:1, :HTS], AF.Exp)
                    nc.vector.tensor_mul(
                        g_exp, g_exp, recip_row[:, s * HTS : (s + 1) * HTS]
                    )
                    nc.vector.tensor_mul(
                        g_exp, g_exp, mask_sb[:, s * HTS : (s + 1) * HTS]
                    )
                    nc.vector.reduce_sum(
                        loss_acc[:, s : s + 1], g_exp, axis=mybir.AxisListType.X
                    )
                    # per-token reciprocal column: transposed view [w, i] of
                    # the 8 contributors' den rows, reduced along free dim
                    dgT = fw.tile([128, NCORES], bf16, tag="dgT")
                    post_reads.append(nc.sync.dma_start(
                        out=dgT[0:HTS, :],
                        in_=rs_outs[s][:, 256 * HTS : 257 * HTS].rearrange(
                            "i w -> w i"
                        ),
                    ))
                    den_cs = fw.tile([128, 1], f32, tag="den_cs")
                    recip_s = fw.tile([128, 1], f32, tag="recip_s")
                    nc.vector.reduce_sum(
                        den_cs[0:HTS], dgT[0:HTS, :], axis=mybir.AxisListType.X
                    )
                    nc.vector.tensor_scalar_add(den_cs[0:HTS], den_cs[0:HTS], -PAD_ENT)
                    nc.vector.reciprocal(recip_s[0:HTS], den_cs[0:HTS])
                    # y rows for this slice
                    for n0 in range(0, D_EMB, 384):
                        po = fp.tile([128, 384], f32, tag="po")
                        for j in range(2):
                            nc.tensor.matmul(
                                po[:HTS, :],
                                pick_bf[:, j * TS + s * HTS : j * TS + (s + 1) * HTS],
                                wbt_sb[:, j * D_EMB + n0 : j * D_EMB + n0 + 384],
                                start=(j == 0),
                                stop=False,
                            )
                        nc.tensor.matmul(
                            po[:HTS, :],
                            onesrow[0:1, 0:HTS],
                            wbb_sb[0:1, n0 : n0 + 384],
                            start=False,
                            stop=True,
                        )
                        y_sb = fw.tile([128, 384], f32, tag="y_sb")
                        nc.vector.tensor_scalar_mul(
                            y_sb[:HTS, :], po[:HTS, :], recip_s[0:HTS, 0:1]
                        )
                        nc.sync.dma_start(
                            out=y_out[s * HTS : (s + 1) * HTS, n0 : n0 + 384],
                            in_=y_sb[:HTS, :],
                        )
                loss_sb = res.tile([1, 1], f32)
                nc.vector.reduce_sum(loss_sb, loss_acc, axis=mybir.AxisListType.X)
                nc.sync.dma_start(out=loss_out[:, :], in_=loss_sb)
            last_scatter = all_scatters[-1]
            for rd in post_reads:
                add_dep_helper(
                    rd.ins, last_scatter.ins, sync=False,
                    reason="post-A2A reads behind final scatter (queue HoL)",
                )

    nc.compile()
    return nc


def _chunked(x, kdim):
    """[kdim*128, F] -> [128, kdim*F] partition-major swizzle."""
    kd128, F = x.shape
    assert kd128 == kdim * 128
    return np.ascontiguousarray(
        x.reshape(kdim, 128, F).transpose(1, 0, 2).reshape(128, kdim * F)
    )


def _prep_host(X, bio, ents, Wf_w, Wf_b, E_w, Wb_w, Wb_b):
    inner = bio == INNER
    run = np.zeros((B, S), np.int64)
    run[:, S - 1] = inner[:, S - 1]
    for t in range(S - 2, -1, -1):
        run[:, t] = np.where(inner[:, t], run[:, t + 1] + 1, 0)
    run_next = np.concatenate([run[:, 1:], np.zeros((B, 1), np.int64)], axis=1)
    end_idx = np.arange(S, dtype=np.int64)[None, :] + run_next

    bb, ss = np.nonzero(bio == BEGIN)
    nb = bb.size
    T = max(128, ((nb + 127) // 128) * 128)
    TS = T // NCORES

    A = np.zeros((T, 2 * D_EMB), np.float32)
    if nb:
        A[:nb, :D_EMB] = X[bb, ss]
        A[:nb, D_EMB:] = X[bb, end_idx[bb, ss]]
    a_t = _chunked(np.ascontiguousarray(A.T), KA).astype(ml_dtypes.bfloat16)
    wf_t = _chunked(np.ascontiguousarray(Wf_w.T), KA).astype(ml_dtypes.bfloat16)
    wf_b = np.ascontiguousarray(Wf_b.reshape(2, 128).T).astype(np.float32)
    wb_t = _chunked(np.ascontiguousarray(Wb_w.T), 2).astype(ml_dtypes.bfloat16)
    wb_b = Wb_b.reshape(1, D_EMB).astype(ml_dtypes.bfloat16)

    ent_ids = ents[bb, ss] if nb else np.zeros((0,), np.int64)
    E_sel = np.zeros((D_ENT, T), np.float32)
    if nb:
        E_sel[:, :nb] = E_w[:, ent_ids]

    in_maps = []
    for c in range(NCORES):
        shard = np.zeros((D_ENT, NS), np.float32)
        shard[:, :SHARD] = E_w[:, c * SHARD : (c + 1) * SHARD]
        e_sh = _chunked(shard, 2).astype(ml_dtypes.bfloat16)
        e_sht = _chunked(np.ascontiguousarray(shard.T), NCH).astype(np.float16)
        esel_c = np.zeros((D_ENT, T), np.float32)
        lo, hi = c * TS, min((c + 1) * TS, nb)
        if hi > lo:
            esel_c[:, lo:hi] = E_sel[:, lo:hi]
        e_sel = _chunked(esel_c, 2).astype(ml_dtypes.bfloat16)
        m = np.zeros((1, TS), np.float32)
        if hi > lo:
            m[0, : hi - lo] = 1.0
        in_maps.append(
            {
                "a_t": a_t,
                "wf_t": wf_t,
                "wf_b": wf_b,
                "e_sh": e_sh,
                "e_sht": e_sht,
                "e_sel": e_sel,
                "mask": m,
                "wb_t": wb_t,
                "wb_b": wb_b,
            }
        )
    return in_maps, bb, ss, nb, T


def kernel(X, bio_output, entities_output, k, Wf_w, Wf_b, E_w, Wb_w, Wb_b):
    global LAST_RESULTS
    from concourse.bass_utils import run_bass_kernel_spmd

    X = np.asarray(X, np.float32)
    bio = np.asarray(bio_output).astype(np.int64)
    ents = np.asarray(entities_output).astype(np.int64)
    Wf_w = np.asarray(Wf_w, np.float32)
    Wf_b = np.asarray(Wf_b, np.float32)
    E_w = np.asarray(E_w, np.float32)
    Wb_w = np.asarray(Wb_w, np.float32)
    Wb_b = np.asarray(Wb_b, np.float32)

    in_maps, bb, ss, nb, T = _prep_host(X, bio, ents, Wf_w, Wf_b, E_w, Wb_w, Wb_b)

    if T not in _cache:
        _cache[T] = _build(T)
    nc = _cache[T]

    res = run_bass_kernel_spmd(
        nc, in_maps, core_ids=list(range(NCORES)), trace=TRACE
    )
    LAST_RESULTS = res

    y_full = np.concatenate([res.results[c]["y_out"] for c in range(NCORES)], axis=0)
    y = np.zeros((B, S, D_EMB), np.float32)
    if nb:
        y[bb, ss] = y_full[:nb]
    loss = np.zeros((1,), np.float32)
    loss[0] = sum(float(res.results[c]["loss_out"][0, 0]) for c in range(NCORES))
    return loss, y
